# revision 3
# baseline (speedup 1.0000x reference)
"""Trainium2 Bass kernel for nn_ATTModule_44865228374086 (moe_routing).

Reference computation (per batch element b):
    pooled = mean(features[b], over H*W)                       # [C]
    h      = relu(pooled @ fc1_w[a] + fc1_b[a])                # [A, CH]
    expert = h @ fc2_w[a] + fc2_b[a]                           # [A, C]
    gate   = softmax(fc_w @ pooled + fc_b)                     # [A]
    mixed  = sum_a gate[a] * expert[a]                         # [C]
    out[b] = features[b] * (1 + sigmoid(mixed))                # [C, H, W]

Strategy: data-parallel over batch across 8 cores (128 samples each).
Per core, features are streamed HBM->SBUF once in groups of G=8 samples,
pooled on DVE (free-axis reduce), the tiny MLP runs on PE in bf16
(weights are host-permuted / pre-scaled by 1/196 and kept SBUF-resident),
the per-channel scale 1+sigmoid(mixed) = 1.5 + 0.5*tanh(mixed/2) is
computed on ACT (tanh lives in the same ACT table set as exp/relu/copy,
so there is a single table load), and the scale is applied in place
(ACT activation-copy / DVE tensor_scalar) before streaming back out.

SBUF layout: partition p holds channels 8p..8p+7, so each per-sample DMA
is 128 partitions x 6272 contiguous bytes.
"""

import numpy as np
import ml_dtypes

import concourse.bacc as bacc
import concourse.tile as tile
from concourse import mybir
from concourse.bass_utils import run_bass_kernel_spmd
from concourse.masks import make_identity

B, C, H, W = 1024, 1024, 14, 14
HW = H * W            # 196
A = 8                 # experts
CH = C // 4           # 256
ACH = A * CH          # 2048
NCORES = 8
BS = B // NCORES      # 128 samples per core
K8 = C // 128         # 8 channels per partition
G = 8                 # samples per MLP group
TPB = 2               # samples per feature tile (one DMA each way)
NT = BS // TPB        # 64 tiles
TPG = G // TPB        # 4 tiles per group
NG = BS // G          # 16 groups
FBUFS = 9             # feature-tile pool depth
ACT_TILES_PER_GROUP = 3  # of TPG tiles, how many apply on ACT (rest on DVE)

F32 = mybir.dt.float32
BF16 = mybir.dt.bfloat16
AF = mybir.ActivationFunctionType
ALU = mybir.AluOpType
AX = mybir.AxisListType

_NC_CACHE = {}


def _emit(tc, nc, feat, w1, wg, w2, b1h, b1g, b2, out):
    # DRAM views: partition p <- channels 8p..8p+7
    fv = feat.rearrange("b (p k) hw -> p b k hw", k=K8)   # [128, BS, 8, 196]
    ov = out.rearrange("b (p k) hw -> p b k hw", k=K8)

    with (
        tc.tile_pool(name="pw", bufs=1) as pw,
        tc.tile_pool(name="pf", bufs=FBUFS) as pf,
        tc.tile_pool(name="pp", bufs=2) as pp,
        tc.tile_pool(name="ph", bufs=2) as ph,
        tc.tile_pool(name="psh", bufs=1, space="PSUM") as psh,
        tc.tile_pool(name="psm", bufs=1, space="PSUM") as psm,
        tc.tile_pool(name="pss", bufs=2, space="PSUM") as pss,
    ):
        # Resident weights / constants.
        w1_sb = pw.tile([128, K8, ACH], BF16)
        nc.sync.dma_start(out=w1_sb, in_=w1)
        wg_sb = pw.tile([128, K8, A], BF16)
        nc.sync.dma_start(out=wg_sb, in_=wg)
        w2_sb = pw.tile([128, ACH // 128, C], BF16)
        nc.sync.dma_start(out=w2_sb, in_=w2)
        b1h_sb = pw.tile([1, ACH], BF16)
        nc.sync.dma_start(out=b1h_sb, in_=b1h)
        b1g_sb = pw.tile([1, A], BF16)
        nc.sync.dma_start(out=b1g_sb, in_=b1g)
        b2_sb = pw.tile([A, C], BF16)
        nc.sync.dma_start(out=b2_sb, in_=b2)
        id16 = pw.tile([128, 128], BF16)
        make_identity(nc, id16)
        id32 = pw.tile([128, 128], F32)
        make_identity(nc, id32)
        ones16 = pw.tile([1, G], BF16)
        nc.vector.memset(ones16, 1.0)

        for g in range(NG):
            # ---- load + pool ----
            Pg = pp.tile([128, K8 * G], F32, tag="Pg")
            Pview = Pg.rearrange("p (k g) -> p g k", g=G)  # [128, G, 8]
            ftiles = []
            for t in range(TPG):
                b0 = g * G + t * TPB
                ft = pf.tile([128, TPB, K8, HW], F32, tag="ft")
                nc.sync.dma_start(out=ft, in_=fv[:, b0 : b0 + TPB])
                nc.vector.tensor_reduce(
                    out=Pview[:, t * TPB : (t + 1) * TPB, :],
                    in_=ft,
                    axis=AX.X,
                    op=ALU.add,
                )
                ftiles.append(ft)
            P16 = pp.tile([128, K8 * G], BF16, tag="P16")
            nc.vector.tensor_copy(out=P16, in_=Pg)

            # ---- fc1 (+ gating logits): psum_h[b, a*CH+ch], psum_g[b, a] ----
            hps = psh.tile([G, ACH], F32)
            gps = pss.tile([G, A], F32, tag="pst")
            for n in range(ACH // 512):
                for k in range(K8):
                    nc.tensor.matmul(
                        out=hps[:, n * 512 : (n + 1) * 512],
                        lhsT=P16[:, k * G : (k + 1) * G],
                        rhs=w1_sb[:, k, n * 512 : (n + 1) * 512],
                        start=(k == 0),
                        stop=False,
                    )
                nc.tensor.matmul(
                    out=hps[:, n * 512 : (n + 1) * 512],
                    lhsT=ones16,
                    rhs=b1h_sb[:, n * 512 : (n + 1) * 512],
                    start=False,
                    stop=True,
                )
            for k in range(K8):
                nc.tensor.matmul(
                    out=gps,
                    lhsT=P16[:, k * G : (k + 1) * G],
                    rhs=wg_sb[:, k, :],
                    start=(k == 0),
                    stop=False,
                )
            nc.tensor.matmul(out=gps, lhsT=ones16, rhs=b1g_sb, start=False, stop=True)

            # ---- softmax gate over A=8 ----
            ge = ph.tile([G, A], F32, tag="ge")
            nc.scalar.activation(out=ge, in_=gps, func=AF.Exp)
            gs = ph.tile([G, 1], F32, tag="gs")
            nc.vector.tensor_reduce(out=gs, in_=ge, axis=AX.X, op=ALU.add)
            gi = ph.tile([G, 1], F32, tag="gi")
            nc.vector.reciprocal(out=gi, in_=gs)
            gate = ph.tile([G, A], F32, tag="gate")
            nc.vector.tensor_scalar_mul(out=gate, in0=ge, scalar1=gi)

            # ---- h' = gate * relu(h), in bf16 (gate>0 so relu(g*h)=g*relu(h)) ----
            h16 = ph.tile([G, ACH], BF16, tag="h16")
            for a in range(A):
                nc.scalar.activation(
                    out=h16[:, a * CH : (a + 1) * CH],
                    in_=hps[:, a * CH : (a + 1) * CH],
                    func=AF.Relu,
                    scale=gate[:, a : a + 1],
                )

            # gate transposed to [A, G] (bf16) for the fc2_b term
            gtp = pss.tile([A, G], F32, tag="pst")
            nc.tensor.transpose(gtp, gate, id32[0:G, 0:G])
            gt16 = ph.tile([A, G], BF16, tag="gt16")
            nc.vector.tensor_copy(out=gt16, in_=gtp)

            # ---- transpose h' to [(a,ch), b] chunks ----
            hT = pp.tile([128, (ACH // 128) * G], BF16, tag="hT")
            for t in range(ACH // 128):
                tp = pss.tile([128, G], BF16, tag="pst")
                nc.tensor.transpose(tp, h16[:, t * 128 : (t + 1) * 128], id16[0:G, 0:G])
                nc.vector.tensor_copy(out=hT[:, t * G : (t + 1) * G], in_=tp)

            # ---- fc2: mixed[b, c] (+ gate-weighted fc2_b) ----
            mps = psm.tile([G, C], F32)
            for n in range(C // 512):
                for t in range(ACH // 128):
                    nc.tensor.matmul(
                        out=mps[:, n * 512 : (n + 1) * 512],
                        lhsT=hT[:, t * G : (t + 1) * G],
                        rhs=w2_sb[:, t, n * 512 : (n + 1) * 512],
                        start=(t == 0),
                        stop=False,
                    )
                nc.tensor.matmul(
                    out=mps[:, n * 512 : (n + 1) * 512],
                    lhsT=gt16,
                    rhs=b2_sb[:, n * 512 : (n + 1) * 512],
                    start=False,
                    stop=True,
                )

            # ---- scale = 1 + sigmoid(mixed) = 1.5 + 0.5*tanh(mixed/2) ----
            mx = ph.tile([G, C], F32, tag="mx")
            nc.scalar.activation(out=mx, in_=mps, func=AF.Tanh, scale=0.5)
            mview = mx.rearrange("g (p k) -> g k p", k=K8)  # [G, 8, 128]
            scl = pp.tile([128, K8 * G], F32, tag="scl")
            for k in range(K8):
                sp = pss.tile([128, G], F32, tag="pst")
                nc.tensor.transpose(sp, mview[:, k, :], id32[0:G, 0:G])
                nc.vector.tensor_scalar(
                    out=scl[:, k * G : (k + 1) * G],
                    in0=sp,
                    scalar1=0.5,
                    scalar2=1.5,
                    op0=ALU.mult,
                    op1=ALU.add,
                )

            # ---- apply scale in place, store ----
            for t in range(TPG):
                ft = ftiles[t]
                on_act = t < ACT_TILES_PER_GROUP
                for bb in range(TPB):
                    bcol = t * TPB + bb
                    for k in range(K8):
                        sl = ft[:, bb, k, :]
                        s1 = scl[:, k * G + bcol : k * G + bcol + 1]
                        if on_act:
                            nc.scalar.activation(out=sl, in_=sl, func=AF.Copy, scale=s1)
                        else:
                            nc.vector.tensor_scalar_mul(out=sl, in0=sl, scalar1=s1)
                b0 = g * G + t * TPB
                nc.sync.dma_start(out=ov[:, b0 : b0 + TPB], in_=ft)


def build_nc():
    nc = bacc.Bacc(
        "TRN2", target_bir_lowering=False, debug=False, num_devices=NCORES
    )
    feat = nc.dram_tensor("features", [BS, C, HW], F32, kind="ExternalInput").ap()
    w1 = nc.dram_tensor("w1", [128, K8, ACH], BF16, kind="ExternalInput").ap()
    wg = nc.dram_tensor("wg", [128, K8, A], BF16, kind="ExternalInput").ap()
    w2 = nc.dram_tensor("w2", [128, ACH // 128, C], BF16, kind="ExternalInput").ap()
    b1h = nc.dram_tensor("b1h", [1, ACH], BF16, kind="ExternalInput").ap()
    b1g = nc.dram_tensor("b1g", [1, A], BF16, kind="ExternalInput").ap()
    b2 = nc.dram_tensor("b2", [A, C], BF16, kind="ExternalInput").ap()
    out = nc.dram_tensor("out", [BS, C, HW], F32, kind="ExternalOutput").ap()

    with tile.TileContext(nc) as tc:
        _emit(tc, nc, feat, w1, wg, w2, b1h, b1g, b2, out)
    nc.compile()
    return nc


def _get_nc():
    if "nc" not in _NC_CACHE:
        _NC_CACHE["nc"] = build_nc()
    return _NC_CACHE["nc"]


def prep_host_inputs(features, fc_w, fc_b, fc1_w, fc1_b, fc2_w, fc2_b):
    """Returns (per-core features list, shared weight dict)."""
    bf16 = ml_dtypes.bfloat16
    s = 1.0 / HW  # fold the spatial mean into the pooled-consuming weights

    fc1_w = np.asarray(fc1_w, dtype=np.float32)
    fc_w = np.asarray(fc_w, dtype=np.float32)
    fc2_w = np.asarray(fc2_w, dtype=np.float32)

    # w1[p, k, a*CH+ch] = fc1_w[a, 8p+k, ch] / HW
    w1 = (
        np.transpose(fc1_w, (1, 0, 2)).reshape(C, ACH).reshape(128, K8, ACH) * s
    ).astype(bf16)
    # wg[p, k, a] = fc_w[a, 8p+k] / HW
    wg = (fc_w.T.reshape(128, K8, A) * s).astype(bf16)
    # w2[p, t, c] = fc2_w.reshape(ACH, C)[t*128+p, c]
    w2 = np.ascontiguousarray(
        fc2_w.reshape(ACH, C).reshape(ACH // 128, 128, C).transpose(1, 0, 2)
    ).astype(bf16)
    weights = {
        "w1": np.ascontiguousarray(w1),
        "wg": np.ascontiguousarray(wg),
        "w2": w2,
        "b1h": np.asarray(fc1_b, dtype=np.float32).reshape(1, ACH).astype(bf16),
        "b1g": np.asarray(fc_b, dtype=np.float32).reshape(1, A).astype(bf16),
        "b2": np.asarray(fc2_b, dtype=np.float32).astype(bf16),
    }
    f = np.ascontiguousarray(np.asarray(features, dtype=np.float32)).reshape(B, C, HW)
    shards = [f[i * BS : (i + 1) * BS] for i in range(NCORES)]
    return shards, weights


def run(inputs, trace=False, trace_kwargs=None):
    nc = _get_nc()
    shards, weights = prep_host_inputs(**inputs)
    in_maps = [dict(weights, features=shards[i]) for i in range(NCORES)]
    res = run_bass_kernel_spmd(
        nc,
        in_maps,
        core_ids=list(range(NCORES)),
        trace=trace,
        **(trace_kwargs or {}),
    )
    out = np.concatenate([res.results[i]["out"] for i in range(NCORES)], axis=0)
    return out.reshape(B, C, H, W), res


def kernel(**inputs):
    out, _ = run(inputs, trace=False)
    return np.asarray(out, dtype=np.float32)


# revision 30
# speedup vs baseline: 111.5165x; 111.5165x over previous
"""Trainium2 Bass kernel for nn_ATTModule_44865228374086 (moe_routing).

Reference computation (per batch element b):
    pooled = mean(features[b], over H*W)                       # [C]
    h      = relu(pooled @ fc1_w[a] + fc1_b[a])                # [A, CH]
    expert = h @ fc2_w[a] + fc2_b[a]                           # [A, C]
    gate   = softmax(fc_w @ pooled + fc_b)                     # [A]
    mixed  = sum_a gate[a] * expert[a]                         # [C]
    out[b] = features[b] * (1 + sigmoid(mixed))                # [C, H, W]

Strategy: data-parallel over batch across 8 cores (128 samples each).
Per core, features are streamed HBM->SBUF once in groups of G=8 samples,
pooled on DVE (free-axis reduce), the tiny MLP runs on PE in bf16
(weights are host-permuted / pre-scaled by 1/196 and kept SBUF-resident),
the per-channel scale 1+sigmoid(mixed) = 1.5 + 0.5*tanh(mixed/2) is
computed on ACT (tanh lives in the same ACT table set as exp/relu/copy,
so there is a single table load), and the scale is applied in place
(ACT activation-copy / DVE tensor_scalar) before streaming back out.

SBUF layout: partition p holds channels 8p..8p+7, so each per-sample DMA
is 128 partitions x 6272 contiguous bytes.

Biases are all zero for this problem (spec fill: zeros); the default
program omits them. If nonzero biases are ever passed, a second program
variant that adds them via K=1 matmuls is built on the fly.
"""

import numpy as np
import ml_dtypes

import concourse.bacc as bacc
import concourse.tile as tile
from concourse import mybir
from concourse.bass_utils import axon_active, run_bass_kernel_spmd
from concourse.masks import make_identity

B, C, H, W = 1024, 1024, 14, 14
HW = H * W            # 196
A = 8                 # experts
CH = C // 4           # 256
ACH = A * CH          # 2048
NCORES = 8
BS = B // NCORES      # 128 samples per core
K8 = C // 128         # 8 channels per partition
G = 8                 # samples per MLP group
TPB = 2               # samples per feature tile (one DMA each way)
NT = BS // TPB        # 64 tiles
TPG = G // TPB        # 4 tiles per group
NG = BS // G          # 16 groups
FBUFS = 10            # feature-tile pool depth
DVE_APPLY_TILES = (0, 1, 2)  # which tiles of a group apply on DVE (rest ACT)
TPB_COPY = 4          # hT transposes batched per PSUM->SBUF copy
COLTILE = False       # column-tile fc2 across PE strips (HW-neutral)
PB_STORES = True      # store per sample instead of per tile

F32 = mybir.dt.float32
BF16 = mybir.dt.bfloat16
AF = mybir.ActivationFunctionType
ALU = mybir.AluOpType
AX = mybir.AxisListType

_NC_CACHE = {}


def _emit(tc, nc, feat, w1, wg, w2, b1h, b1g, b2, out, with_bias, repeat=1):
    # DRAM views: partition p <- channels 8p..8p+7
    fv = feat.rearrange("b (p k) hw -> p b k hw", k=K8)   # [128, BS, 8, 196]
    ov = out.rearrange("b (p k) hw -> p b k hw", k=K8)

    with (
        tc.tile_pool(name="pw", bufs=1) as pw,
        tc.tile_pool(name="pf", bufs=FBUFS) as pf,
        tc.tile_pool(name="pp", bufs=2) as pp,
        tc.tile_pool(name="ph1", bufs=1) as ph1,
        tc.tile_pool(name="ph2", bufs=2) as ph2,
        tc.tile_pool(name="psh", bufs=1, space="PSUM") as psh,
        tc.tile_pool(name="psm", bufs=1, space="PSUM") as psm,
        tc.tile_pool(name="pss", bufs=2, space="PSUM") as pss,
    ):
        # Resident weights / constants.
        w1_sb = pw.tile([128, K8, ACH], BF16)
        nc.scalar.dma_start(out=w1_sb, in_=w1)
        wg_sb = pw.tile([128, K8, A], BF16)
        nc.scalar.dma_start(out=wg_sb, in_=wg)
        w2_sb = pw.tile([128, ACH // 128, C], BF16)
        nc.scalar.dma_start(out=w2_sb, in_=w2)
        id16 = pw.tile([128, 128], BF16)
        make_identity(nc, id16)
        if with_bias:
            id32 = pw.tile([128, 128], F32)
            make_identity(nc, id32)
            b1h_sb = pw.tile([1, ACH], BF16)
            nc.sync.dma_start(out=b1h_sb, in_=b1h)
            b1g_sb = pw.tile([1, A], BF16)
            nc.sync.dma_start(out=b1g_sb, in_=b1g)
            b2_sb = pw.tile([A, C], BF16)
            nc.sync.dma_start(out=b2_sb, in_=b2)
            ones16 = pw.tile([1, G], BF16)
            nc.vector.memset(ones16, 1.0)

        import contextlib
        loop_cm = tc.For_i(0, repeat, 1) if repeat > 1 else contextlib.nullcontext()
        with loop_cm:
            _emit_groups(
                tc, nc, fv, ov, pf, pp, ph1, ph2, psh, psm, pss,
                w1_sb, wg_sb, w2_sb, id16,
                (id32, b1h_sb, b1g_sb, b2_sb, ones16) if with_bias else None,
                with_bias,
            )


def _emit_groups(tc, nc, fv, ov, pf, pp, ph1, ph2, psh, psm, pss,
                 w1_sb, wg_sb, w2_sb, id16, bias_tiles, with_bias):
        if with_bias:
            id32, b1h_sb, b1g_sb, b2_sb, ones16 = bias_tiles
        for g in range(NG):
            # ---- load + pool ----
            Pg = pp.tile([128, K8 * G], F32, tag="Pg")
            Pview = Pg.rearrange("p (k g) -> p g k", g=G)  # [128, G, 8]
            ftiles = []
            for t in range(TPG):
                b0 = g * G + t * TPB
                ft = pf.tile([128, TPB, K8, HW], F32, tag="ft")
                nc.sync.dma_start(out=ft, in_=fv[:, b0 : b0 + TPB])
                # Per-sample reduces: smaller DVE quanta schedule around the
                # latency-critical MLP ops instead of blocking them.
                for bb in range(TPB):
                    nc.vector.tensor_reduce(
                        out=Pview[:, t * TPB + bb : t * TPB + bb + 1, :],
                        in_=ft[:, bb],
                        axis=AX.X,
                        op=ALU.add,
                    )
                ftiles.append(ft)
            P16 = pp.tile([128, K8 * G], BF16, tag="P16")
            nc.vector.tensor_copy(out=P16, in_=Pg)

            # ---- gating logits first (softmax overlaps the fc1 matmuls) ----
            gps = pss.tile([G, A], F32, tag="pst")
            for k in range(K8):
                nc.tensor.matmul(
                    out=gps,
                    lhsT=P16[:, k * G : (k + 1) * G],
                    rhs=wg_sb[:, k, :],
                    start=(k == 0),
                    stop=(k == K8 - 1) and not with_bias,
                )
            if with_bias:
                nc.tensor.matmul(
                    out=gps, lhsT=ones16, rhs=b1g_sb, start=False, stop=True
                )

            # ---- fc1: psum_h[b, a*CH+ch] ----
            hps = psh.tile([G, ACH], F32)
            for n in range(ACH // 512):
                for k in range(K8):
                    nc.tensor.matmul(
                        out=hps[:, n * 512 : (n + 1) * 512],
                        lhsT=P16[:, k * G : (k + 1) * G],
                        rhs=w1_sb[:, k, n * 512 : (n + 1) * 512],
                        start=(k == 0),
                        stop=(k == K8 - 1) and not with_bias,
                    )
                if with_bias:
                    nc.tensor.matmul(
                        out=hps[:, n * 512 : (n + 1) * 512],
                        lhsT=ones16,
                        rhs=b1h_sb[:, n * 512 : (n + 1) * 512],
                        start=False,
                        stop=True,
                    )

            # ---- softmax gate over A=8 (ACT/DVE, hides under fc1) ----
            ge = ph2.tile([G, A], F32, tag="ge")
            nc.scalar.activation(out=ge, in_=gps, func=AF.Exp)
            gs = ph2.tile([G, 1], F32, tag="gs")
            nc.vector.tensor_reduce(out=gs, in_=ge, axis=AX.X, op=ALU.add)
            gi = ph2.tile([G, 1], F32, tag="gi")
            nc.vector.reciprocal(out=gi, in_=gs)
            gate = ph2.tile([G, A], F32, tag="gate")
            nc.vector.tensor_scalar_mul(out=gate, in0=ge, scalar1=gi)

            if with_bias:
                gtp = pss.tile([A, G], F32, tag="pst")
                nc.tensor.transpose(gtp, gate, id32[0:G, 0:G])
                gt16 = ph2.tile([A, G], BF16, tag="gt16")
                nc.vector.tensor_copy(out=gt16, in_=gtp)

            # ---- h' = gate*relu(h) -> transpose -> fc2, pipelined per chunk.
            # PE executes in emission order, so interleave the transposes with
            # the fc2 matmuls; relu (ACT) and psum->sbuf copies (DVE) overlap.
            h16 = ph1.tile([G, ACH], BF16, tag="h16")
            hT = pp.tile([128, (ACH // 128) * G], BF16, tag="hT")
            # fc2 is column-tiled: the two 512-col output banks go to PE
            # column strips at partitions 0 and 32 and run concurrently.
            if COLTILE:
                mps = psm.tile([40, 512], F32, tag="mps")
            else:
                mps = psm.tile([G, C], F32, tag="mps")
            nt = ACH // 128
            for t0 in range(0, nt, TPB_COPY):
                for t in range(t0, t0 + TPB_COPY):
                    if t % 2 == 0:
                        a = t // 2
                        nc.scalar.activation(
                            out=h16[:, a * CH : (a + 1) * CH],
                            in_=hps[:, a * CH : (a + 1) * CH],
                            func=AF.Relu,
                            scale=gate[:, a : a + 1],
                        )
                tp = pss.tile([128, TPB_COPY * G], BF16, tag="pst")
                for j in range(TPB_COPY):
                    nc.tensor.transpose(
                        tp[:, j * G : (j + 1) * G],
                        h16[:, (t0 + j) * 128 : (t0 + j + 1) * 128],
                        id16[0:G, 0:G],
                    )
                nc.vector.tensor_copy(
                    out=hT[:, t0 * G : (t0 + TPB_COPY) * G], in_=tp
                )
                for t in range(t0, t0 + TPB_COPY):
                    for n in range(C // 512):
                        out_ap = (
                            mps[32 * n : 32 * n + G, :]
                            if COLTILE
                            else mps[:, n * 512 : (n + 1) * 512]
                        )
                        nc.tensor.matmul(
                            out=out_ap,
                            lhsT=hT[:, t * G : (t + 1) * G],
                            rhs=w2_sb[:, t, n * 512 : (n + 1) * 512],
                            start=(t == 0),
                            stop=(t == nt - 1) and not with_bias,
                            tile_position=(0, 32 * n) if COLTILE else None,
                            skip_group_check=COLTILE,
                        )
            if with_bias:
                for n in range(C // 512):
                    out_ap = (
                        mps[32 * n : 32 * n + G, :]
                        if COLTILE
                        else mps[:, n * 512 : (n + 1) * 512]
                    )
                    nc.tensor.matmul(
                        out=out_ap,
                        lhsT=gt16,
                        rhs=b2_sb[:, n * 512 : (n + 1) * 512],
                        start=False,
                        stop=True,
                        tile_position=(0, 32 * n) if COLTILE else None,
                        skip_group_check=COLTILE,
                    )

            # ---- scale = 1 + sigmoid(mixed) = 1.5 + 0.5*tanh(mixed/2) ----
            scl = pp.tile([128, K8 * G], F32, tag="scl")
            sp = pss.tile([128, K8 * G], BF16, tag="pst")
            if COLTILE:
                mx = ph1.tile([40, 512], BF16, tag="mx")
                for n in range(C // 512):
                    nc.scalar.activation(
                        out=mx[32 * n : 32 * n + G, :],
                        in_=mps[32 * n : 32 * n + G, :],
                        func=AF.Tanh,
                        scale=0.5,
                    )
                # mx strip n, col c' = tanh(mixed[b, 512n + c']/2); channel
                # c = 8p+k lives at (strip p//64, col 8*(p%64)+k).
                mxv = mx.rearrange("p (q k) -> p k q", k=K8)  # [40, 8, 64]
                for k in range(K8):
                    nc.tensor.transpose(
                        sp[0:64, k * G : (k + 1) * G],
                        mxv[0:G, k, :],
                        id16[0:G, 0:G],
                        tile_position=(0, 0),
                    )
                    nc.tensor.transpose(
                        sp[64:128, k * G : (k + 1) * G],
                        mxv[32 : 32 + G, k, :],
                        id16[32 : 32 + G, 32 : 32 + G],
                        tile_position=(32, 64),
                    )
            else:
                mx = ph1.tile([G, C], BF16, tag="mx")
                nc.scalar.activation(out=mx, in_=mps, func=AF.Tanh, scale=0.5)
                mxv = mx.rearrange("g (p k) -> g k p", k=K8)  # [G, 8, 128]
                for k in range(K8):
                    nc.tensor.transpose(
                        sp[:, k * G : (k + 1) * G], mxv[:, k, :], id16[0:G, 0:G]
                    )
            nc.vector.tensor_scalar(
                out=scl,
                in0=sp,
                scalar1=0.5,
                scalar2=1.5,
                op0=ALU.mult,
                op1=ALU.add,
            )

            # ---- apply scale in place, store ----
            for t in range(TPG):
                ft = ftiles[t]
                on_dve = t in DVE_APPLY_TILES
                for bb in range(TPB):
                    bcol = t * TPB + bb
                    for k in range(K8):
                        sl = ft[:, bb, k, :]
                        s1 = scl[:, k * G + bcol : k * G + bcol + 1]
                        if on_dve:
                            nc.vector.tensor_scalar_mul(out=sl, in0=sl, scalar1=s1)
                        else:
                            nc.scalar.activation(out=sl, in_=sl, func=AF.Copy, scale=s1)
                    if PB_STORES:
                        b0 = g * G + t * TPB + bb
                        nc.sync.dma_start(out=ov[:, b0], in_=ft[:, bb])
                if not PB_STORES:
                    b0 = g * G + t * TPB
                    nc.sync.dma_start(out=ov[:, b0 : b0 + TPB], in_=ft)


def build_nc(with_bias=False, repeat=1):
    nc = bacc.Bacc(
        "TRN2",
        target_bir_lowering=False,
        debug=not axon_active(),
        num_devices=NCORES,
    )
    feat = nc.dram_tensor("features", [BS, C, HW], F32, kind="ExternalInput").ap()
    w1 = nc.dram_tensor("w1", [128, K8, ACH], BF16, kind="ExternalInput").ap()
    wg = nc.dram_tensor("wg", [128, K8, A], BF16, kind="ExternalInput").ap()
    w2 = nc.dram_tensor("w2", [128, ACH // 128, C], BF16, kind="ExternalInput").ap()
    b1h = b1g = b2 = None
    if with_bias:
        b1h = nc.dram_tensor("b1h", [1, ACH], BF16, kind="ExternalInput").ap()
        b1g = nc.dram_tensor("b1g", [1, A], BF16, kind="ExternalInput").ap()
        b2 = nc.dram_tensor("b2", [A, C], BF16, kind="ExternalInput").ap()
    out = nc.dram_tensor("out", [BS, C, HW], F32, kind="ExternalOutput").ap()

    with tile.TileContext(nc) as tc:
        _emit(tc, nc, feat, w1, wg, w2, b1h, b1g, b2, out, with_bias, repeat)
    nc.compile()
    return nc


def _get_nc(with_bias=False):
    key = "bias" if with_bias else "nobias"
    if key not in _NC_CACHE:
        _NC_CACHE[key] = build_nc(with_bias)
    return _NC_CACHE[key]


def prep_host_inputs(features, fc_w, fc_b, fc1_w, fc1_b, fc2_w, fc2_b):
    """Returns (per-core features list, shared weight dict, with_bias)."""
    bf16 = ml_dtypes.bfloat16
    s = 1.0 / HW  # fold the spatial mean into the pooled-consuming weights

    fc1_w = np.asarray(fc1_w, dtype=np.float32)
    fc_w = np.asarray(fc_w, dtype=np.float32)
    fc2_w = np.asarray(fc2_w, dtype=np.float32)
    fc_b = np.asarray(fc_b, dtype=np.float32)
    fc1_b = np.asarray(fc1_b, dtype=np.float32)
    fc2_b = np.asarray(fc2_b, dtype=np.float32)
    with_bias = bool(np.any(fc_b) or np.any(fc1_b) or np.any(fc2_b))

    # w1[p, k, a*CH+ch] = fc1_w[a, 8p+k, ch] / HW
    w1 = (
        np.transpose(fc1_w, (1, 0, 2)).reshape(C, ACH).reshape(128, K8, ACH) * s
    ).astype(bf16)
    # wg[p, k, a] = fc_w[a, 8p+k] / HW
    wg = (fc_w.T.reshape(128, K8, A) * s).astype(bf16)
    # w2[p, t, c] = fc2_w.reshape(ACH, C)[t*128+p, c]
    w2 = np.ascontiguousarray(
        fc2_w.reshape(ACH, C).reshape(ACH // 128, 128, C).transpose(1, 0, 2)
    ).astype(bf16)
    weights = {
        "w1": np.ascontiguousarray(w1),
        "wg": np.ascontiguousarray(wg),
        "w2": w2,
    }
    if with_bias:
        weights["b1h"] = fc1_b.reshape(1, ACH).astype(bf16)
        weights["b1g"] = fc_b.reshape(1, A).astype(bf16)
        weights["b2"] = fc2_b.astype(bf16)
    f = np.ascontiguousarray(np.asarray(features, dtype=np.float32)).reshape(B, C, HW)
    shards = [f[i * BS : (i + 1) * BS] for i in range(NCORES)]
    return shards, weights, with_bias


def run(inputs, trace=False, trace_kwargs=None):
    shards, weights, with_bias = prep_host_inputs(**inputs)
    nc = _get_nc(with_bias)
    in_maps = [dict(weights, features=shards[i]) for i in range(NCORES)]
    res = run_bass_kernel_spmd(
        nc,
        in_maps,
        core_ids=list(range(NCORES)),
        trace=trace,
        **(trace_kwargs or {}),
    )
    out = np.concatenate([res.results[i]["out"] for i in range(NCORES)], axis=0)
    return out.reshape(B, C, H, W), res


def kernel(**inputs):
    out, _ = run(inputs, trace=False)
    return np.asarray(out, dtype=np.float32)
